# revision 38
# baseline (speedup 1.0000x reference)
"""v4: bf16 MHA, ACT-saturating schedule.

Design (per core: one batch b = c//2, head-group g = c%2 of E=512 dims):
- Emission is a single interleaved stream built around the attention
  (QK^T -> exp -> attnV) pipeline, which is ScalarE(exp)-bound at
  ~1.04us per 2-head j-tile. All other PE work (Q/K/V projections,
  out-projection, normalization broadcasts) is emitted as small filler
  units pumped between attention steps so it executes in TensorE slack.
- Startup: xq/xk/xv staged on three DMA queues (Pool/DVE/SP); K-proj(p0)
  + Q-proj(p0,ic0) emitted first so the first exp fires ~15us in.
- V-projection is split per head-pair (N=128 matmuls) and emitted
  just-in-time inside the ic=0 attention blocks.
- PSUM: s_ps double-buffer (4 banks) + a_ps pair (2) + work pool (2,
  shared by v/p/o/bc tiles) = 8 banks exactly.
- QK^T pair uses lhsT base partitions 0/64 -> tile_position (0,0)/(64,0)
  row tiling; on HW the two K=64 matmuls run concurrently in the PE.
Measured numpy-model precision: rel_rms ~6.6e-3 vs the f32 reference.
"""

from collections import deque

import numpy as np
import ml_dtypes

import concourse.bass as bass
import concourse.mybir as mybir
import concourse.tile as tile
from concourse import bacc
from concourse.bass_utils import run_bass_kernel_spmd

B, S, D = 4, 2048, 1024
HT, DK = 16, 64
G = 2
NCORES = 8
E = D // G
H = HT // G
EC = E // 128
KD = D // 128
SM = S // 128
SN = S // 512
F32 = mybir.dt.float32
BF16 = mybir.dt.bfloat16
NPBF16 = ml_dtypes.bfloat16
EXP = mybir.ActivationFunctionType.Exp

REPS = 1


def _build_mha_nc(reps=1):
    nc = bacc.Bacc("TRN2", target_bir_lowering=False, debug=False)

    xq = nc.dram_tensor("xq_t", [D, S], BF16, kind="ExternalInput")
    xk = nc.dram_tensor("xk_t", [D, S], BF16, kind="ExternalInput")
    xv = nc.dram_tensor("xv_t", [D, S], BF16, kind="ExternalInput")
    wq = nc.dram_tensor("wq_t", [D, E], BF16, kind="ExternalInput")
    wk = nc.dram_tensor("wk_t", [D, E], BF16, kind="ExternalInput")
    wv = nc.dram_tensor("wv_t", [D, E], BF16, kind="ExternalInput")
    wo = nc.dram_tensor("wo_t", [E, D], BF16, kind="ExternalInput")
    bq = nc.dram_tensor("b_q", [E], F32, kind="ExternalInput")
    bk = nc.dram_tensor("b_k", [E], F32, kind="ExternalInput")
    out = nc.dram_tensor("out", [S, D], F32, kind="ExternalOutput")

    with tile.TileContext(nc) as tc:
        for _ in range(reps):
            _mha_body(tc, xq, xk, xv, wq, wk, wv, wo, bq, bk, out)
    nc.compile()
    return nc


class _Pump:
    """Filler-work queue: units are generators yielding ~matmul-sized chunks."""

    def __init__(self):
        self.q = deque()

    def push(self, due, gen):
        self.q.append([due, gen])

    def pump(self, budget):
        while budget > 0 and self.q:
            due, g = self.q[0]
            try:
                budget -= next(g)
            except StopIteration:
                self.q.popleft()

    def drain_due(self, key):
        kept = deque()
        while self.q:
            due, g = self.q.popleft()
            if due is not None and due <= key:
                for _ in g:
                    pass
            else:
                kept.append([due, g])
        self.q = kept

    def drain_all(self):
        while self.q:
            _, g = self.q.popleft()
            for _ in g:
                pass


def _mha_body(tc, xq, xk, xv, wq, wk, wv, wo, bq, bk, out):
    nc = tc.nc
    from contextlib import ExitStack

    with ExitStack() as ctx:
        singles = ctx.enter_context(tc.tile_pool(name="singles", bufs=1))
        persist = ctx.enter_context(tc.tile_pool(name="persist", bufs=1))
        ps_s = ctx.enter_context(tc.tile_pool(name="ps_s", bufs=2, space="PSUM"))
        ps_a = ctx.enter_context(tc.tile_pool(name="ps_a", bufs=2, space="PSUM"))
        ps_w = ctx.enter_context(tc.tile_pool(name="ps_w", bufs=2, space="PSUM"))

        ones_row = singles.tile([128, 128], BF16)
        nc.vector.memset(ones_row, 1.0)
        bqc = singles.tile([128, EC], F32)
        bkc = singles.tile([128, EC], F32)
        for c in range(EC):
            nc.sync.dma_start(out=bqc[:, c : c + 1], in_=bq[c * 128 : (c + 1) * 128])
            nc.sync.dma_start(out=bkc[:, c : c + 1], in_=bk[c * 128 : (c + 1) * 128])

        qT = [persist.tile([128, S], BF16, name=f"qT{c}") for c in range(EC)]
        kT = [persist.tile([128, S], BF16, name=f"kT{c}") for c in range(EC)]
        v_aug = [persist.tile([128, H * 65], BF16, name=f"vaug{s}") for s in range(SM)]
        aT = [persist.tile([128, S], BF16, name=f"aT{p}") for p in range(EC)]
        wo_sb = [persist.tile([128, D], BF16, name=f"wo{d}") for d in range(EC)]

        x_pool = ctx.enter_context(tc.tile_pool(name="x_sb", bufs=1))
        xq_sb = [x_pool.tile([128, S], BF16, name=f"xq{d}") for d in range(KD)]
        wq_sb = [x_pool.tile([128, E], BF16, name=f"wq{d}") for d in range(KD)]
        e_pool = ctx.enter_context(tc.tile_pool(name="e_sb", bufs=4))
        nrm_pool = ctx.enter_context(tc.tile_pool(name="nrm", bufs=1))
        late_pools = {}

        kv_ctx = ExitStack()
        kv_pool = kv_ctx.enter_context(tc.tile_pool(name="kv_sb", bufs=1))
        xk_sb = [kv_pool.tile([128, S], BF16, name=f"xk{d}") for d in range(KD)]
        xv_sb = [kv_pool.tile([128, S], BF16, name=f"xv{d}") for d in range(KD)]
        wk_sb = [kv_pool.tile([128, E], BF16, name=f"wk{d}") for d in range(KD)]
        wv_sb = [kv_pool.tile([128, E], BF16, name=f"wv{d}") for d in range(KD)]

        # --- input staging on three queues, critical tensors first ---
        for d in range(KD):
            nc.gpsimd.dma_start(out=xq_sb[d], in_=xq[d * 128 : (d + 1) * 128, :])
        for d in range(4):
            nc.scalar.dma_start(out=xk_sb[d], in_=xk[d * 128 : (d + 1) * 128, :])
        for d in range(2):
            nc.scalar.dma_start(out=xv_sb[d], in_=xv[d * 128 : (d + 1) * 128, :])
        for d in range(KD):
            nc.scalar.dma_start(out=wv_sb[d], in_=wv[d * 128 : (d + 1) * 128, :])
        for d in range(KD):
            nc.sync.dma_start(out=wk_sb[d], in_=wk[d * 128 : (d + 1) * 128, :])
        for d in range(KD):
            nc.sync.dma_start(out=wq_sb[d], in_=wq[d * 128 : (d + 1) * 128, :])
        for d in range(4, KD):
            nc.sync.dma_start(out=xk_sb[d], in_=xk[d * 128 : (d + 1) * 128, :])
        for d in range(2, 4):
            nc.sync.dma_start(out=xv_sb[d], in_=xv[d * 128 : (d + 1) * 128, :])
        for d in range(4, KD):
            nc.gpsimd.dma_start(out=xv_sb[d], in_=xv[d * 128 : (d + 1) * 128, :])
        for d in range(EC):
            nc.sync.dma_start(out=wo_sb[d], in_=wo[d * 128 : (d + 1) * 128, :])

        def kproj_unit(p, s):
            p_ps = ps_w.tile([128, 512], F32, name="p_ps", tag="w")
            for d in range(KD):
                nc.tensor.matmul(
                    p_ps,
                    wk_sb[d][:, p * 128 : (p + 1) * 128],
                    xk_sb[d][:, s * 512 : (s + 1) * 512],
                    start=(d == 0),
                    stop=(d == KD - 1),
                )
                yield 1
            nc.vector.tensor_scalar_add(
                kT[p][:, s * 512 : (s + 1) * 512], p_ps, bkc[:, p : p + 1]
            )
            yield 0

        def qproj_unit(p, ic):
            p_ps = ps_w.tile([128, 512], F32, name="p_ps", tag="w")
            for d in range(KD):
                nc.tensor.matmul(
                    p_ps,
                    wq_sb[d][:, p * 128 : (p + 1) * 128],
                    xq_sb[d][:, ic * 512 : (ic + 1) * 512],
                    start=(d == 0),
                    stop=(d == KD - 1),
                )
                yield 1
            nc.vector.tensor_scalar_add(
                qT[p][:, ic * 512 : (ic + 1) * 512], p_ps, bqc[:, p : p + 1]
            )
            yield 0

        def outproj_unit(ic, s, e2):
            o_ps = ps_w.tile([128, 512], F32, name="o_ps", tag="w")
            for d in range(EC):
                nc.tensor.matmul(
                    o_ps,
                    aT[d][:, s * 128 : (s + 1) * 128],
                    wo_sb[d][:, e2 * 512 : (e2 + 1) * 512],
                    start=(d == 0),
                    stop=(d == EC - 1),
                )
                yield 1
            o_sb = late_pools["out"].tile([128, 512], F32, name="o_sb", tag="o")
            nc.vector.tensor_copy(o_sb, o_ps)
            nc.sync.dma_start(
                out=out[s * 128 : (s + 1) * 128, e2 * 512 : (e2 + 1) * 512],
                in_=o_sb,
            )
            yield 0

        def vproj_pp(half, j):
            # project V columns for head-pairs {2*half, 2*half+1}, key-tile j
            v_ps = ps_w.tile([128, 256], F32, name="v_ps", tag="w")
            for d in range(KD):
                nc.tensor.matmul(
                    v_ps,
                    xv_sb[d][:, j * 128 : (j + 1) * 128],
                    wv_sb[d][:, half * 256 : (half + 1) * 256],
                    start=(d == 0),
                    stop=(d == KD - 1),
                )
            va = v_aug[j].rearrange("q (h w) -> q h w", w=65)
            if half == 0:
                nc.vector.memset(va[:, :, 64:65], 1.0)
            nc.vector.tensor_copy(
                va[:, 4 * half : 4 * half + 4, 0:64],
                v_ps.rearrange("q (h w) -> q h w", w=64),
            )

        pump = _Pump()

        def make_tail(p, ic, a_ps0, a_ps1, e_last):
            # trailing work of block (p, ic): last attnV pair + normalization.
            # Emitted just after the NEXT block's first QK/exp so the exp
            # stream never waits behind it.
            i0 = ic * 512
            h0, h1 = 2 * p, 2 * p + 1

            def tail():
                nc.tensor.matmul(
                    a_ps0,
                    v_aug[SM - 1][:, h0 * 65 : h0 * 65 + 65],
                    e_last[:, 0:512],
                    start=False,
                    stop=True,
                )
                nc.tensor.matmul(
                    a_ps1,
                    v_aug[SM - 1][:, h1 * 65 : h1 * 65 + 65],
                    e_last[:, 512:1024],
                    start=False,
                    stop=True,
                )
                for hh, a_ps in ((0, a_ps0), (1, a_ps1)):
                    rec = nrm_pool.tile([65, 512], BF16, name="rec", tag="rec")
                    with nc.allow_low_precision(
                        reason="softmax denom reciprocal; bf16 err ~4e-3 verified"
                    ):
                        nc.vector.reciprocal(rec[64:65, :], a_ps[64:65, :])
                    bc_ps = ps_w.tile([64, 512], F32, name="bc_ps", tag="w")
                    nc.tensor.matmul(
                        bc_ps,
                        ones_row[64:65, 0:64],
                        rec[64:65, :],
                        start=True,
                        stop=True,
                    )
                    bc_sb = nrm_pool.tile([64, 512], BF16, name="bc_sb", tag="bc")
                    with nc.allow_low_precision(
                        reason="softmax denom broadcast; bf16 err ~4e-3 verified"
                    ):
                        nc.vector.tensor_copy(bc_sb, bc_ps)
                    if hh == 0:
                        nc.vector.tensor_mul(
                            aT[p][0:64, i0 : i0 + 512], a_ps[0:64, :], bc_sb
                        )
                    else:
                        tmp = nrm_pool.tile([64, 512], BF16, name="tmp", tag="tmp")
                        nc.vector.tensor_mul(tmp, a_ps[0:64, :], bc_sb)
                        nc.gpsimd.dma_start(
                            out=aT[p][64:128, i0 : i0 + 512], in_=tmp
                        )
                pump.pump(4)

            return tail

        def block(p, ic, prev_tail):
            i0 = ic * 512
            h0, h1 = 2 * p, 2 * p + 1
            pump.drain_due((ic, p))
            a_ps0 = ps_a.tile([65, 512], F32, name="a_ps0", tag="a")
            a_ps1 = ps_a.tile([65, 512], F32, name="a_ps1", tag="a")
            e_tiles = {}
            for j in range(SM):
                s_ps = ps_s.tile([128, 1024], F32, name="s_ps", tag="s")
                with tc.high_priority():
                    nc.tensor.matmul(
                        s_ps[:, 0:512],
                        kT[p][0:64, j * 128 : (j + 1) * 128],
                        qT[p][0:64, i0 : i0 + 512],
                        start=True,
                        stop=True,
                    )
                    nc.tensor.matmul(
                        s_ps[:, 512:1024],
                        kT[p][64:128, j * 128 : (j + 1) * 128],
                        qT[p][64:128, i0 : i0 + 512],
                        start=True,
                        stop=True,
                    )
                ep = e_pool if ic == 0 else late_pools["e2"]
                e0 = ep.tile([128, 1024], BF16, name="e0", tag="e")
                nc.scalar.activation(e0, s_ps, EXP, scale=0.125)
                e_tiles[j] = e0
                if j == 0 and prev_tail is not None:
                    prev_tail()
                if j > 0:
                    ej = e_tiles.pop(j - 1)
                    nc.tensor.matmul(
                        a_ps0,
                        v_aug[j - 1][:, h0 * 65 : h0 * 65 + 65],
                        ej[:, 0:512],
                        start=(j - 1 == 0),
                        stop=False,
                    )
                    nc.tensor.matmul(
                        a_ps1,
                        v_aug[j - 1][:, h1 * 65 : h1 * 65 + 65],
                        ej[:, 512:1024],
                        start=(j - 1 == 0),
                        stop=False,
                    )
                if ic == 0 and p < 2:
                    vproj_pp(p, j)
                pump.pump(2 if ic > 0 else 4)
            return make_tail(p, ic, a_ps0, a_ps1, e_tiles.pop(SM - 1))

        # --- prologue: projections for the first block ---
        for _ in kproj_unit(0, 0):
            pass
        for _ in qproj_unit(0, 0):
            pass
        for s in range(1, SN):
            pump.push((0, 0.5), kproj_unit(0, s))

        # --- main loop ---
        tail = None
        for ic in range(SN):
            if ic > 0:
                for s in range(4):
                    for e2 in range(2):
                        pump.push(None, outproj_unit(ic - 1, (ic - 1) * 4 + s, e2))
            for p in range(EC):
                if ic == 0 and p + 1 < EC:
                    for s in range(SN):
                        pump.push((0, p + 1), kproj_unit(p + 1, s))
                    pump.push((0, p + 1), qproj_unit(p + 1, 0))
                if ic == 0 and p == EC - 1:
                    for pp in range(EC):
                        pump.push((1, pp), qproj_unit(pp, 1))
                if ic > 0 and ic + 1 < SN:
                    pump.push((ic + 1, p), qproj_unit(p, ic + 1))
                tail = block(p, ic, tail)
            if ic == 0:
                # K/V staging no longer needed; reuse the SBUF for deeper
                # e-buffering and the output staging tiles.
                kv_ctx.close()
                late_pools["e2"] = ctx.enter_context(
                    tc.tile_pool(name="e2_sb", bufs=6)
                )
                late_pools["out"] = ctx.enter_context(
                    tc.tile_pool(name="osb", bufs=3)
                )
        tail()
        pump.drain_all()
        for s in range(4):
            for e2 in range(2):
                for _ in outproj_unit(SN - 1, (SN - 1) * 4 + s, e2):
                    pass


def _prep_in_maps(query, key, value, w_q, b_q, w_k, b_k, w_v, b_v, w_o):
    f32 = np.float32
    in_maps = []
    for c in range(NCORES):
        b, g = c // G, c % G
        sl = slice(g * E, (g + 1) * E)
        in_maps.append(
            {
                "xq_t": np.ascontiguousarray(query[b].T).astype(NPBF16),
                "xk_t": np.ascontiguousarray(key[b].T).astype(NPBF16),
                "xv_t": np.ascontiguousarray(value[b].T).astype(NPBF16),
                "wq_t": np.ascontiguousarray(w_q[sl, :].T).astype(NPBF16),
                "wk_t": np.ascontiguousarray(w_k[sl, :].T).astype(NPBF16),
                "wv_t": np.ascontiguousarray(w_v[sl, :].T).astype(NPBF16),
                "wo_t": np.ascontiguousarray(w_o[:, sl].T).astype(NPBF16),
                "b_q": np.ascontiguousarray(b_q[sl], dtype=f32),
                "b_k": np.ascontiguousarray(b_k[sl], dtype=f32),
            }
        )
    return in_maps


_NC_CACHE = {}


def run(inputs, trace=False, **kw):
    if REPS not in _NC_CACHE:
        _NC_CACHE[REPS] = _build_mha_nc(REPS)
    nc = _NC_CACHE[REPS]
    in_maps = _prep_in_maps(
        inputs["query"], inputs["key"], inputs["value"],
        inputs["w_q"], inputs["b_q"], inputs["w_k"], inputs["b_k"],
        inputs["w_v"], inputs["b_v"], inputs["w_o"],
    )
    res = run_bass_kernel_spmd(nc, in_maps, list(range(NCORES)), trace=trace, **kw)
    bias_vec = (
        np.asarray(inputs["b_o"], dtype=np.float32)
        + np.asarray(inputs["w_o"], dtype=np.float32)
        @ np.asarray(inputs["b_v"], dtype=np.float32)
    )
    full = np.empty((B, S, D), dtype=np.float32)
    for b in range(B):
        full[b] = res.results[2 * b]["out"] + res.results[2 * b + 1]["out"] + bias_vec
    return full, res


def kernel(**inputs):
    full, _ = run(inputs)
    return full


def _make_timed_callable(nc, in_maps):
    import jax
    from jax.sharding import Mesh, PartitionSpec
    from jax.experimental.shard_map import shard_map
    from concourse import bass2jax, mybir as mb

    partition_name = nc.partition_id_tensor.name if nc.partition_id_tensor else None
    in_names, out_names, out_avals, zero_outs = [], [], [], []
    for alloc in nc.m.functions[0].allocations:
        if not isinstance(alloc, mb.MemoryLocationSet):
            continue
        name = alloc.memorylocations[0].name
        if alloc.kind == "ExternalInput":
            if name != partition_name:
                in_names.append(name)
        elif alloc.kind == "ExternalOutput":
            out_names.append(name)
            shape = tuple(alloc.tensor_shape)
            dtype = mb.dt.np(alloc.dtype)
            out_avals.append(jax.core.ShapedArray(shape, dtype))
            zero_outs.append(np.zeros(shape, dtype))
    n_params = len(in_names)
    in_names = in_names + out_names
    if partition_name is not None:
        in_names.append(partition_name)
    donate = tuple(range(n_params, n_params + len(out_names)))

    def _body(*args):
        operands = list(args)
        if partition_name is not None:
            operands.append(bass2jax.partition_id_tensor())
        outs = bass2jax._bass_exec_p.bind(
            *operands,
            out_avals=tuple(out_avals),
            in_names=tuple(in_names),
            out_names=tuple(out_names),
            lowering_input_output_aliases=(),
            sim_require_finite=True,
            sim_require_nnan=True,
            nc=nc,
        )
        return tuple(outs)

    devices = jax.devices()[:NCORES]
    mesh = Mesh(np.asarray(devices).reshape(NCORES), ("core",))
    in_specs = (PartitionSpec("core"),) * (n_params + len(out_names))
    out_specs = (PartitionSpec("core"),) * len(out_names)
    sharded = jax.jit(
        shard_map(_body, mesh=mesh, in_specs=in_specs, out_specs=out_specs,
                  check_rep=False),
        donate_argnums=donate, keep_unused=True,
    )
    concat_in = [
        np.concatenate([in_maps[c][in_names[i]] for c in range(NCORES)], axis=0)
        for i in range(n_params)
    ]
    dev_in = [jax.device_put(a) for a in concat_in]

    def call():
        zeros_dev = [
            jax.device_put(np.zeros((NCORES * z.shape[0], *z.shape[1:]), z.dtype))
            for z in zero_outs
        ]
        jax.block_until_ready(zeros_dev)
        import time

        t0 = time.perf_counter()
        out_arrs = sharded(*dev_in, *zeros_dev)
        jax.block_until_ready(out_arrs)
        dt = time.perf_counter() - t0
        return out_arrs, dt

    def gather(out_arrs):
        return [
            {
                name: np.asarray(out_arrs[i]).reshape(NCORES, *out_avals[i].shape)[c]
                for i, name in enumerate(out_names)
            }
            for c in range(NCORES)
        ]

    return call, gather


def run_timed(inputs, iters=6):
    """Measure device execution via repeated pjrt calls (amortizes RPC)."""
    global REPS
    from concourse import bass2jax

    bass2jax.install_neuronx_cc_hook()
    in_maps = _prep_in_maps(
        inputs["query"], inputs["key"], inputs["value"],
        inputs["w_q"], inputs["b_q"], inputs["w_k"], inputs["b_k"],
        inputs["w_v"], inputs["b_v"], inputs["w_o"],
    )
    if REPS not in _NC_CACHE:
        _NC_CACHE[REPS] = _build_mha_nc(REPS)
    call, gather = _make_timed_callable(_NC_CACHE[REPS], in_maps)
    times = []
    out_arrs = None
    for _ in range(iters):
        out_arrs, dt = call()
        times.append(dt)
    res = gather(out_arrs)
    bias_vec = (
        np.asarray(inputs["b_o"], dtype=np.float32)
        + np.asarray(inputs["w_o"], dtype=np.float32)
        @ np.asarray(inputs["b_v"], dtype=np.float32)
    )
    full = np.empty((B, S, D), dtype=np.float32)
    for b in range(B):
        full[b] = res[2 * b]["out"] + res[2 * b + 1]["out"] + bias_vec
    return full, times


def run_timed_pair(inputs, nrep=9, iters=24):
    """Interleave REPS=1 and REPS=nrep executions so slow wall-clock drift
    cancels in the per-iteration delta. Returns (full_output_r1, deltas_s)
    where deltas[i] = t_rn[i] - t_r1[i]; kernel time ~= min(deltas)/(nrep-1).
    """
    global REPS
    from concourse import bass2jax

    bass2jax.install_neuronx_cc_hook()
    in_maps = _prep_in_maps(
        inputs["query"], inputs["key"], inputs["value"],
        inputs["w_q"], inputs["b_q"], inputs["w_k"], inputs["b_k"],
        inputs["w_v"], inputs["b_v"], inputs["w_o"],
    )
    old = REPS
    try:
        for r in (1, nrep):
            REPS = r
            if r not in _NC_CACHE:
                _NC_CACHE[r] = _build_mha_nc(r)
    finally:
        REPS = old
    call1, gather1 = _make_timed_callable(_NC_CACHE[1], in_maps)
    calln, gathern = _make_timed_callable(_NC_CACHE[nrep], in_maps)
    out1, _ = call1()
    outn, _ = calln()  # compile+warm both
    t1s, tns = [], []
    for _ in range(iters):
        out1, dt1 = call1()
        outn, dtn = calln()
        t1s.append(dt1)
        tns.append(dtn)
    res1, resn = gather1(out1), gathern(outn)
    for c in range(NCORES):
        assert np.allclose(res1[c]["out"], resn[c]["out"], atol=1e-5)
    bias_vec = (
        np.asarray(inputs["b_o"], dtype=np.float32)
        + np.asarray(inputs["w_o"], dtype=np.float32)
        @ np.asarray(inputs["b_v"], dtype=np.float32)
    )
    full = np.empty((B, S, D), dtype=np.float32)
    for b in range(B):
        full[b] = res1[2 * b]["out"] + res1[2 * b + 1]["out"] + bias_vec
    return full, t1s, tns


# revision 46
# speedup vs baseline: 1.4927x; 1.4927x over previous
"""v10: bf16 MHA, ACT-saturating interleaved schedule.

Design (per core: one batch b = c//2, head-group g = c%2 of E=512 dims):
- Emission is a single interleaved stream built around the attention
  (QK^T -> exp -> attnV) pipeline, which is ScalarE(exp)-bound at
  ~1.04us per 2-head j-tile. All other PE work (Q/K/V projections,
  out-projection, normalization broadcasts) is emitted as generator
  "filler units" pumped between attention steps so it executes in
  TensorE slack; per-unit due-keys force-drain anything the next block
  needs.
- Cross-block software pipelining: the next block's first QK/exp are
  emitted before the previous block's last attnV + normalization (tail
  closure), and the QK^T pair is wrapped in tc.high_priority() so PE
  never drains filler backlog ahead of the exp stream's producer.
- Startup: inputs staged across Pool/ACT/SP DMA queues ordered by first
  use; K-proj(p0) + Q-proj(p0,ic0) emitted first so the first exp fires
  ~16us in.
- V-projection is computed just-in-time per head-pair-pair (N=256
  units) inside the ic=0 attention blocks.
- PSUM: s_ps double-buffer (4 banks) + a_ps pair (2) + work pool (2,
  shared by v/p/o/bc tiles) = 8 banks exactly. SBUF: the xk/xv staging
  pool closes after ic0 emission, funding an 8-deep e-tile pool that
  absorbs the ~5us serial normalization chain at block handoffs.
- QK^T pair uses lhsT base partitions 0/64 -> tile_position (0,0)/(64,0)
  row tiling; on HW the two K=64 matmuls run concurrently in the PE.
- Softmax denominator rides as the 65th column of the augmented-V
  stationary (attnV M=65); the reciprocal row is broadcast to 64
  partitions with a K=1 ones-matmul (gpsimd partition_broadcast is
  broken on this HW build - NaN despite passing CoreSim).
CoreSim span 477us (v3 baseline) -> 383us. HW-verified rel_rms 6.795e-3.
"""

from collections import deque

import numpy as np
import ml_dtypes

import concourse.bass as bass
import concourse.mybir as mybir
import concourse.tile as tile
from concourse import bacc
from concourse.bass_utils import run_bass_kernel_spmd

B, S, D = 4, 2048, 1024
HT, DK = 16, 64
G = 2
NCORES = 8
E = D // G
H = HT // G
EC = E // 128
KD = D // 128
SM = S // 128
SN = S // 512
F32 = mybir.dt.float32
BF16 = mybir.dt.bfloat16
NPBF16 = ml_dtypes.bfloat16
EXP = mybir.ActivationFunctionType.Exp

REPS = 1


def _build_mha_nc(reps=1):
    nc = bacc.Bacc("TRN2", target_bir_lowering=False, debug=False)

    xq = nc.dram_tensor("xq_t", [D, S], BF16, kind="ExternalInput")
    xk = nc.dram_tensor("xk_t", [D, S], BF16, kind="ExternalInput")
    xv = nc.dram_tensor("xv_t", [D, S], BF16, kind="ExternalInput")
    wq = nc.dram_tensor("wq_t", [D, E], BF16, kind="ExternalInput")
    wk = nc.dram_tensor("wk_t", [D, E], BF16, kind="ExternalInput")
    wv = nc.dram_tensor("wv_t", [D, E], BF16, kind="ExternalInput")
    wo = nc.dram_tensor("wo_t", [E, D], BF16, kind="ExternalInput")
    bq = nc.dram_tensor("b_q", [E], F32, kind="ExternalInput")
    bk = nc.dram_tensor("b_k", [E], F32, kind="ExternalInput")
    out = nc.dram_tensor("out", [S, D], F32, kind="ExternalOutput")

    with tile.TileContext(nc) as tc:
        for _ in range(reps):
            _mha_body(tc, xq, xk, xv, wq, wk, wv, wo, bq, bk, out)
    nc.compile()
    return nc


class _Pump:
    """Filler-work queue: units are generators yielding ~matmul-sized chunks."""

    def __init__(self):
        self.q = deque()

    def push(self, due, gen):
        self.q.append([due, gen])

    def pump(self, budget):
        while budget > 0 and self.q:
            due, g = self.q[0]
            try:
                budget -= next(g)
            except StopIteration:
                self.q.popleft()

    def drain_due(self, key):
        kept = deque()
        while self.q:
            due, g = self.q.popleft()
            if due is not None and due <= key:
                for _ in g:
                    pass
            else:
                kept.append([due, g])
        self.q = kept

    def drain_all(self):
        while self.q:
            _, g = self.q.popleft()
            for _ in g:
                pass


def _mha_body(tc, xq, xk, xv, wq, wk, wv, wo, bq, bk, out):
    nc = tc.nc
    from contextlib import ExitStack

    with ExitStack() as ctx:
        singles = ctx.enter_context(tc.tile_pool(name="singles", bufs=1))
        persist = ctx.enter_context(tc.tile_pool(name="persist", bufs=1))
        ps_s = ctx.enter_context(tc.tile_pool(name="ps_s", bufs=2, space="PSUM"))
        ps_a = ctx.enter_context(tc.tile_pool(name="ps_a", bufs=2, space="PSUM"))
        ps_w = ctx.enter_context(tc.tile_pool(name="ps_w", bufs=2, space="PSUM"))

        ones_row = singles.tile([128, 128], BF16)
        nc.vector.memset(ones_row, 1.0)
        bqc = singles.tile([128, EC], F32)
        bkc = singles.tile([128, EC], F32)
        for c in range(EC):
            nc.sync.dma_start(out=bqc[:, c : c + 1], in_=bq[c * 128 : (c + 1) * 128])
            nc.sync.dma_start(out=bkc[:, c : c + 1], in_=bk[c * 128 : (c + 1) * 128])

        qT = [persist.tile([128, S], BF16, name=f"qT{c}") for c in range(EC)]
        kT = [persist.tile([128, S], BF16, name=f"kT{c}") for c in range(EC)]
        v_aug = [persist.tile([128, H * 65], BF16, name=f"vaug{s}") for s in range(SM)]
        aT = [persist.tile([128, S], BF16, name=f"aT{p}") for p in range(EC)]
        wo_sb = [persist.tile([128, D], BF16, name=f"wo{d}") for d in range(EC)]

        x_pool = ctx.enter_context(tc.tile_pool(name="x_sb", bufs=1))
        xq_sb = [x_pool.tile([128, S], BF16, name=f"xq{d}") for d in range(KD)]
        wq_sb = [x_pool.tile([128, E], BF16, name=f"wq{d}") for d in range(KD)]
        e_pool = ctx.enter_context(tc.tile_pool(name="e_sb", bufs=4))
        nrm_pool = ctx.enter_context(tc.tile_pool(name="nrm", bufs=1))
        late_pools = {}

        kv_ctx = ExitStack()
        kv_pool = kv_ctx.enter_context(tc.tile_pool(name="kv_sb", bufs=1))
        xk_sb = [kv_pool.tile([128, S], BF16, name=f"xk{d}") for d in range(KD)]
        xv_sb = [kv_pool.tile([128, S], BF16, name=f"xv{d}") for d in range(KD)]
        wk_sb = [kv_pool.tile([128, E], BF16, name=f"wk{d}") for d in range(KD)]
        wv_sb = [kv_pool.tile([128, E], BF16, name=f"wv{d}") for d in range(KD)]

        # --- input staging on three queues, critical tensors first ---
        for d in range(KD):
            nc.gpsimd.dma_start(out=xq_sb[d], in_=xq[d * 128 : (d + 1) * 128, :])
        for d in range(4):
            nc.scalar.dma_start(out=xk_sb[d], in_=xk[d * 128 : (d + 1) * 128, :])
        for d in range(2):
            nc.scalar.dma_start(out=xv_sb[d], in_=xv[d * 128 : (d + 1) * 128, :])
        for d in range(KD):
            nc.scalar.dma_start(out=wv_sb[d], in_=wv[d * 128 : (d + 1) * 128, :])
        for d in range(4, KD):
            nc.sync.dma_start(out=xk_sb[d], in_=xk[d * 128 : (d + 1) * 128, :])
        for d in range(KD):
            nc.sync.dma_start(out=wk_sb[d], in_=wk[d * 128 : (d + 1) * 128, :])
            nc.sync.dma_start(out=wq_sb[d], in_=wq[d * 128 : (d + 1) * 128, :])
        for d in range(2, 4):
            nc.sync.dma_start(out=xv_sb[d], in_=xv[d * 128 : (d + 1) * 128, :])
        for d in range(4, KD):
            nc.gpsimd.dma_start(out=xv_sb[d], in_=xv[d * 128 : (d + 1) * 128, :])
        for d in range(EC):
            nc.sync.dma_start(out=wo_sb[d], in_=wo[d * 128 : (d + 1) * 128, :])

        def kproj_unit(p, s):
            p_ps = ps_w.tile([128, 512], F32, name="p_ps", tag="w")
            for d in range(KD):
                nc.tensor.matmul(
                    p_ps,
                    wk_sb[d][:, p * 128 : (p + 1) * 128],
                    xk_sb[d][:, s * 512 : (s + 1) * 512],
                    start=(d == 0),
                    stop=(d == KD - 1),
                )
                yield 1
            nc.vector.tensor_scalar_add(
                kT[p][:, s * 512 : (s + 1) * 512], p_ps, bkc[:, p : p + 1]
            )
            yield 0

        def qproj_unit(p, ic):
            p_ps = ps_w.tile([128, 512], F32, name="p_ps", tag="w")
            for d in range(KD):
                nc.tensor.matmul(
                    p_ps,
                    wq_sb[d][:, p * 128 : (p + 1) * 128],
                    xq_sb[d][:, ic * 512 : (ic + 1) * 512],
                    start=(d == 0),
                    stop=(d == KD - 1),
                )
                yield 1
            nc.vector.tensor_scalar_add(
                qT[p][:, ic * 512 : (ic + 1) * 512], p_ps, bqc[:, p : p + 1]
            )
            yield 0

        def outproj_unit(ic, s, e2):
            o_ps = ps_w.tile([128, 512], F32, name="o_ps", tag="w")
            for d in range(EC):
                nc.tensor.matmul(
                    o_ps,
                    aT[d][:, s * 128 : (s + 1) * 128],
                    wo_sb[d][:, e2 * 512 : (e2 + 1) * 512],
                    start=(d == 0),
                    stop=(d == EC - 1),
                )
                yield 1
            o_sb = late_pools["out"].tile([128, 512], F32, name="o_sb", tag="o")
            nc.vector.tensor_copy(o_sb, o_ps)
            nc.sync.dma_start(
                out=out[s * 128 : (s + 1) * 128, e2 * 512 : (e2 + 1) * 512],
                in_=o_sb,
            )
            yield 0

        def vproj_pp(half, j):
            # project V columns for head-pairs {2*half, 2*half+1}, key-tile j
            v_ps = ps_w.tile([128, 256], F32, name="v_ps", tag="w")
            for d in range(KD):
                nc.tensor.matmul(
                    v_ps,
                    xv_sb[d][:, j * 128 : (j + 1) * 128],
                    wv_sb[d][:, half * 256 : (half + 1) * 256],
                    start=(d == 0),
                    stop=(d == KD - 1),
                )
            va = v_aug[j].rearrange("q (h w) -> q h w", w=65)
            if half == 0:
                nc.vector.memset(va[:, :, 64:65], 1.0)
            nc.vector.tensor_copy(
                va[:, 4 * half : 4 * half + 4, 0:64],
                v_ps.rearrange("q (h w) -> q h w", w=64),
            )

        pump = _Pump()

        def make_tail(p, ic, a_ps0, a_ps1, e_last):
            # trailing work of block (p, ic): last attnV pair + normalization.
            # Emitted just after the NEXT block's first QK/exp so the exp
            # stream never waits behind it.
            i0 = ic * 512
            h0, h1 = 2 * p, 2 * p + 1

            def tail():
                nc.tensor.matmul(
                    a_ps0,
                    v_aug[SM - 1][:, h0 * 65 : h0 * 65 + 65],
                    e_last[:, 0:512],
                    start=False,
                    stop=True,
                )
                nc.tensor.matmul(
                    a_ps1,
                    v_aug[SM - 1][:, h1 * 65 : h1 * 65 + 65],
                    e_last[:, 512:1024],
                    start=False,
                    stop=True,
                )
                for hh, a_ps in ((0, a_ps0), (1, a_ps1)):
                    rec = nrm_pool.tile([65, 512], BF16, name="rec", tag="rec")
                    with nc.allow_low_precision(
                        reason="softmax denom reciprocal; bf16 err ~4e-3 verified"
                    ):
                        nc.vector.reciprocal(rec[64:65, :], a_ps[64:65, :])
                    bc_ps = ps_w.tile([64, 512], F32, name="bc_ps", tag="w")
                    nc.tensor.matmul(
                        bc_ps,
                        ones_row[64:65, 0:64],
                        rec[64:65, :],
                        start=True,
                        stop=True,
                    )
                    bc_sb = nrm_pool.tile([64, 512], BF16, name="bc_sb", tag="bc")
                    with nc.allow_low_precision(
                        reason="softmax denom broadcast; bf16 err ~4e-3 verified"
                    ):
                        nc.vector.tensor_copy(bc_sb, bc_ps)
                    if hh == 0:
                        nc.vector.tensor_mul(
                            aT[p][0:64, i0 : i0 + 512], a_ps[0:64, :], bc_sb
                        )
                    else:
                        tmp = nrm_pool.tile([64, 512], BF16, name="tmp", tag="tmp")
                        nc.vector.tensor_mul(tmp, a_ps[0:64, :], bc_sb)
                        nc.gpsimd.dma_start(
                            out=aT[p][64:128, i0 : i0 + 512], in_=tmp
                        )
                pump.pump(4)

            return tail

        def block(p, ic, prev_tail):
            i0 = ic * 512
            h0, h1 = 2 * p, 2 * p + 1
            pump.drain_due((ic, p))
            a_ps0 = ps_a.tile([65, 512], F32, name="a_ps0", tag="a")
            a_ps1 = ps_a.tile([65, 512], F32, name="a_ps1", tag="a")
            e_tiles = {}
            for j in range(SM):
                s_ps = ps_s.tile([128, 1024], F32, name="s_ps", tag="s")
                with tc.high_priority():
                    nc.tensor.matmul(
                        s_ps[:, 0:512],
                        kT[p][0:64, j * 128 : (j + 1) * 128],
                        qT[p][0:64, i0 : i0 + 512],
                        start=True,
                        stop=True,
                    )
                    nc.tensor.matmul(
                        s_ps[:, 512:1024],
                        kT[p][64:128, j * 128 : (j + 1) * 128],
                        qT[p][64:128, i0 : i0 + 512],
                        start=True,
                        stop=True,
                    )
                ep = e_pool if ic == 0 else late_pools["e2"]
                e0 = ep.tile([128, 1024], BF16, name="e0", tag="e")
                nc.scalar.activation(e0, s_ps, EXP, scale=0.125)
                e_tiles[j] = e0
                if j == 0 and prev_tail is not None:
                    prev_tail()
                if j > 0:
                    ej = e_tiles.pop(j - 1)
                    nc.tensor.matmul(
                        a_ps0,
                        v_aug[j - 1][:, h0 * 65 : h0 * 65 + 65],
                        ej[:, 0:512],
                        start=(j - 1 == 0),
                        stop=False,
                    )
                    nc.tensor.matmul(
                        a_ps1,
                        v_aug[j - 1][:, h1 * 65 : h1 * 65 + 65],
                        ej[:, 512:1024],
                        start=(j - 1 == 0),
                        stop=False,
                    )
                if ic == 0 and p < 2:
                    vproj_pp(p, j)
                pump.pump(2 if ic > 0 else 4)
            return make_tail(p, ic, a_ps0, a_ps1, e_tiles.pop(SM - 1))

        # --- prologue: projections for the first block ---
        for _ in kproj_unit(0, 0):
            pass
        for _ in qproj_unit(0, 0):
            pass
        for s in range(1, SN):
            pump.push((0, 0.5), kproj_unit(0, s))

        # --- main loop ---
        tail = None
        for ic in range(SN):
            if ic > 0:
                for s in range(4):
                    for e2 in range(2):
                        pump.push(None, outproj_unit(ic - 1, (ic - 1) * 4 + s, e2))
            for p in range(EC):
                if ic == 0 and p + 1 < EC:
                    for s in range(SN):
                        pump.push((0, p + 1), kproj_unit(p + 1, s))
                    pump.push((0, p + 1), qproj_unit(p + 1, 0))
                if ic == 0 and p == EC - 1:
                    for pp in range(EC):
                        pump.push((1, pp), qproj_unit(pp, 1))
                if ic > 0 and ic + 1 < SN:
                    pump.push((ic + 1, p), qproj_unit(p, ic + 1))
                tail = block(p, ic, tail)
            if ic == 0:
                # K/V staging no longer needed; reuse the SBUF for deeper
                # e-buffering and the output staging tiles.
                kv_ctx.close()
                late_pools["e2"] = ctx.enter_context(
                    tc.tile_pool(name="e2_sb", bufs=8)
                )
                late_pools["out"] = ctx.enter_context(
                    tc.tile_pool(name="osb", bufs=4)
                )
        tail()
        pump.drain_all()
        for s in range(4):
            for e2 in range(2):
                for _ in outproj_unit(SN - 1, (SN - 1) * 4 + s, e2):
                    pass


def _prep_in_maps(query, key, value, w_q, b_q, w_k, b_k, w_v, b_v, w_o):
    f32 = np.float32
    in_maps = []
    for c in range(NCORES):
        b, g = c // G, c % G
        sl = slice(g * E, (g + 1) * E)
        in_maps.append(
            {
                "xq_t": np.ascontiguousarray(query[b].T).astype(NPBF16),
                "xk_t": np.ascontiguousarray(key[b].T).astype(NPBF16),
                "xv_t": np.ascontiguousarray(value[b].T).astype(NPBF16),
                "wq_t": np.ascontiguousarray(w_q[sl, :].T).astype(NPBF16),
                "wk_t": np.ascontiguousarray(w_k[sl, :].T).astype(NPBF16),
                "wv_t": np.ascontiguousarray(w_v[sl, :].T).astype(NPBF16),
                "wo_t": np.ascontiguousarray(w_o[:, sl].T).astype(NPBF16),
                "b_q": np.ascontiguousarray(b_q[sl], dtype=f32),
                "b_k": np.ascontiguousarray(b_k[sl], dtype=f32),
            }
        )
    return in_maps


_NC_CACHE = {}


def run(inputs, trace=False, **kw):
    if REPS not in _NC_CACHE:
        _NC_CACHE[REPS] = _build_mha_nc(REPS)
    nc = _NC_CACHE[REPS]
    in_maps = _prep_in_maps(
        inputs["query"], inputs["key"], inputs["value"],
        inputs["w_q"], inputs["b_q"], inputs["w_k"], inputs["b_k"],
        inputs["w_v"], inputs["b_v"], inputs["w_o"],
    )
    res = run_bass_kernel_spmd(nc, in_maps, list(range(NCORES)), trace=trace, **kw)
    bias_vec = (
        np.asarray(inputs["b_o"], dtype=np.float32)
        + np.asarray(inputs["w_o"], dtype=np.float32)
        @ np.asarray(inputs["b_v"], dtype=np.float32)
    )
    full = np.empty((B, S, D), dtype=np.float32)
    for b in range(B):
        full[b] = res.results[2 * b]["out"] + res.results[2 * b + 1]["out"] + bias_vec
    return full, res


def kernel(**inputs):
    full, _ = run(inputs)
    return full


def _make_timed_callable(nc, in_maps):
    import jax
    from jax.sharding import Mesh, PartitionSpec
    from jax.experimental.shard_map import shard_map
    from concourse import bass2jax, mybir as mb

    partition_name = nc.partition_id_tensor.name if nc.partition_id_tensor else None
    in_names, out_names, out_avals, zero_outs = [], [], [], []
    for alloc in nc.m.functions[0].allocations:
        if not isinstance(alloc, mb.MemoryLocationSet):
            continue
        name = alloc.memorylocations[0].name
        if alloc.kind == "ExternalInput":
            if name != partition_name:
                in_names.append(name)
        elif alloc.kind == "ExternalOutput":
            out_names.append(name)
            shape = tuple(alloc.tensor_shape)
            dtype = mb.dt.np(alloc.dtype)
            out_avals.append(jax.core.ShapedArray(shape, dtype))
            zero_outs.append(np.zeros(shape, dtype))
    n_params = len(in_names)
    in_names = in_names + out_names
    if partition_name is not None:
        in_names.append(partition_name)
    donate = tuple(range(n_params, n_params + len(out_names)))

    def _body(*args):
        operands = list(args)
        if partition_name is not None:
            operands.append(bass2jax.partition_id_tensor())
        outs = bass2jax._bass_exec_p.bind(
            *operands,
            out_avals=tuple(out_avals),
            in_names=tuple(in_names),
            out_names=tuple(out_names),
            lowering_input_output_aliases=(),
            sim_require_finite=True,
            sim_require_nnan=True,
            nc=nc,
        )
        return tuple(outs)

    devices = jax.devices()[:NCORES]
    mesh = Mesh(np.asarray(devices).reshape(NCORES), ("core",))
    in_specs = (PartitionSpec("core"),) * (n_params + len(out_names))
    out_specs = (PartitionSpec("core"),) * len(out_names)
    sharded = jax.jit(
        shard_map(_body, mesh=mesh, in_specs=in_specs, out_specs=out_specs,
                  check_rep=False),
        donate_argnums=donate, keep_unused=True,
    )
    concat_in = [
        np.concatenate([in_maps[c][in_names[i]] for c in range(NCORES)], axis=0)
        for i in range(n_params)
    ]
    dev_in = [jax.device_put(a) for a in concat_in]

    def call():
        zeros_dev = [
            jax.device_put(np.zeros((NCORES * z.shape[0], *z.shape[1:]), z.dtype))
            for z in zero_outs
        ]
        jax.block_until_ready(zeros_dev)
        import time

        t0 = time.perf_counter()
        out_arrs = sharded(*dev_in, *zeros_dev)
        jax.block_until_ready(out_arrs)
        dt = time.perf_counter() - t0
        return out_arrs, dt

    def gather(out_arrs):
        return [
            {
                name: np.asarray(out_arrs[i]).reshape(NCORES, *out_avals[i].shape)[c]
                for i, name in enumerate(out_names)
            }
            for c in range(NCORES)
        ]

    return call, gather


def run_timed(inputs, iters=6):
    """Measure device execution via repeated pjrt calls (amortizes RPC)."""
    global REPS
    from concourse import bass2jax

    bass2jax.install_neuronx_cc_hook()
    in_maps = _prep_in_maps(
        inputs["query"], inputs["key"], inputs["value"],
        inputs["w_q"], inputs["b_q"], inputs["w_k"], inputs["b_k"],
        inputs["w_v"], inputs["b_v"], inputs["w_o"],
    )
    if REPS not in _NC_CACHE:
        _NC_CACHE[REPS] = _build_mha_nc(REPS)
    call, gather = _make_timed_callable(_NC_CACHE[REPS], in_maps)
    times = []
    out_arrs = None
    for _ in range(iters):
        out_arrs, dt = call()
        times.append(dt)
    res = gather(out_arrs)
    bias_vec = (
        np.asarray(inputs["b_o"], dtype=np.float32)
        + np.asarray(inputs["w_o"], dtype=np.float32)
        @ np.asarray(inputs["b_v"], dtype=np.float32)
    )
    full = np.empty((B, S, D), dtype=np.float32)
    for b in range(B):
        full[b] = res[2 * b]["out"] + res[2 * b + 1]["out"] + bias_vec
    return full, times


def run_timed_pair(inputs, nrep=9, iters=24):
    """Interleave REPS=1 and REPS=nrep executions so slow wall-clock drift
    cancels in the per-iteration delta. Returns (full_output_r1, deltas_s)
    where deltas[i] = t_rn[i] - t_r1[i]; kernel time ~= min(deltas)/(nrep-1).
    """
    global REPS
    from concourse import bass2jax

    bass2jax.install_neuronx_cc_hook()
    in_maps = _prep_in_maps(
        inputs["query"], inputs["key"], inputs["value"],
        inputs["w_q"], inputs["b_q"], inputs["w_k"], inputs["b_k"],
        inputs["w_v"], inputs["b_v"], inputs["w_o"],
    )
    old = REPS
    try:
        for r in (1, nrep):
            REPS = r
            if r not in _NC_CACHE:
                _NC_CACHE[r] = _build_mha_nc(r)
    finally:
        REPS = old
    call1, gather1 = _make_timed_callable(_NC_CACHE[1], in_maps)
    calln, gathern = _make_timed_callable(_NC_CACHE[nrep], in_maps)
    out1, _ = call1()
    outn, _ = calln()  # compile+warm both
    t1s, tns = [], []
    for _ in range(iters):
        out1, dt1 = call1()
        outn, dtn = calln()
        t1s.append(dt1)
        tns.append(dtn)
    res1, resn = gather1(out1), gathern(outn)
    for c in range(NCORES):
        assert np.allclose(res1[c]["out"], resn[c]["out"], atol=1e-5)
    bias_vec = (
        np.asarray(inputs["b_o"], dtype=np.float32)
        + np.asarray(inputs["w_o"], dtype=np.float32)
        @ np.asarray(inputs["b_v"], dtype=np.float32)
    )
    full = np.empty((B, S, D), dtype=np.float32)
    for b in range(B):
        full[b] = res1[2 * b]["out"] + res1[2 * b + 1]["out"] + bias_vec
    return full, t1s, tns
